# revision 27
# baseline (speedup 1.0000x reference)
"""Two-layer GAT (DGL GATConv semantics) on 8 Trainium2 NeuronCores.

Self-contained kernel: kernel(**inputs) takes the FULL inputs from
reference.setup_inputs() and returns the FULL [50000, 768] output
concat([x, h1, h2]).

Strategy (graph-parallel, dst-ownership sharding), v3.2:
- Nodes sharded by dst ownership: core c owns nodes [c*NPC, (c+1)*NPC).
  Edges routed to the dst owner, grouped into chunks of 128 dst nodes.
- Per layer, a DRAM table holds fp32 rows [feat(256) | el(4) | pad] with
  320-f32 row stride (1280B, %256B for dma_gather). Each core computes
  table rows for its OWN nodes only (fp32 matmul; er falls out of the
  same matmul). The table is replicated with GROUPED AllGathers (7 row
  groups per layer, group-major global row ids) so the collectives
  pipeline behind the per-chunk table builds instead of serializing.
- Edge phase per chunk: two bulk dma_gathers of src rows (A/B table
  halves for int16 indices; pad slots gather row 0 and are nullified by
  the match matrices; negative-index skipping and runtime counts CRASH
  the hardware DMA descgen). The ~7ns/row descriptor generation on
  gpsimd is the kernel's critical path, so everything else is kept off
  gpsimd, and DVE work is batched into one whole-chunk op per step
  (per-op overhead dominates small ops): one is_equal builds all 18
  match matrices, one mult builds all feat*exa products, one subtract
  builds all lo-residuals; bf16 downcast copies run on the Scalar
  engine (ACT Copy), attention nonlinearity via ACT Lrelu+Exp.
- Precision: the harness tolerance (2e-2 relative with a 1e-3 floor)
  requires ~2^-14 per-term accuracy in the weighted aggregation — bf16,
  fp16 and fp32r all fail; and layer-2 amplifies layer-1 error, so h1
  needs ~1e-5 absolute accuracy. All value paths (feat, el, er, exa,
  softmax denominator) are fp32 or exact bf16 hi/lo SPLIT PAIRS
  (2^-17): the message matmul uses rhs=[Chi|Clo] (512 cols, one PSUM
  bank) at bf16 speed, er and the denominator use 8-col hi/lo pairs.
- Layer 2: h1^T staged through a DRAM tile (fp32), table built
  shard-wise and AllGathered exactly like layer 1.
"""

import dataclasses
import numpy as np
import ml_dtypes

import concourse.bass as bass
import concourse.bacc as bacc
import concourse.tile as tile
import concourse.mybir as mybir
from concourse.masks import make_identity

F32 = mybir.dt.float32
BF16 = mybir.dt.bfloat16
I16 = mybir.dt.int16
AX = mybir.AxisListType
OP = mybir.AluOpType
ACT = mybir.ActivationFunctionType
NPBF = ml_dtypes.bfloat16

P = 128
D = 256          # feature dim (in and out)
H = 4            # heads
DH = 64          # dim per head
ROWF = 320       # table row stride in f32 elems (1280B, %256B)
RD = 260         # used row cols: 256 feat + 4 el
NEG_SLOPE = 0.2


@dataclasses.dataclass
class Cfg:
    N: int            # real node count
    E: int            # edge count
    NCORES: int = 8
    TA: int = 9       # gather tiles per chunk from table half A
    TB: int = 9       # gather tiles per chunk from table half B
    NG: int = 3       # gather buffer rotation depth

    @property
    def NPC(self):
        return self.N // self.NCORES

    @property
    def NCHUNK(self):
        return (self.NPC + P - 1) // P

    @property
    def NCHP(self):
        # chunk count padded to even: the table splits into two halves
        # (one AllGather + one Shared tile each, matching the A/B gathers)
        return self.NCHUNK + (self.NCHUNK % 2)

    @property
    def LASTC(self):
        return self.NPC - (self.NCHUNK - 1) * P

    @property
    def NPCPAD(self):
        return self.NCHP * P

    @property
    def NPAD(self):
        return self.NCORES * self.NPCPAD

    @property
    def GR(self):
        # rows per table half per core
        return self.NPCPAD // 2

    @property
    def SPLIT(self):
        assert self.NPAD % 2 == 0
        s = self.NPAD // 2
        assert s <= 32767 and self.NPAD - s <= 32767
        return s

    @property
    def T(self):
        return self.TA + self.TB


FULL = Cfg(N=50000, E=800000)

# profiling mode: replace the collective with local copies
PROFILE_LOCAL_CC = False
# phase subset for ablation profiling (None = all)
PHASES = None


def _on(name):
    return PHASES is None or name in PHASES


# ---------------------------------------------------------------- host prep

def _wrap_idx(flat, ntile):
    """Flat int array [ntile*128] -> dma_gather idx sbuf layout [128, ntile*8].
    idx j lives at [j%16, j//16]; replicated across the 8 partition groups."""
    w = flat.reshape(ntile * 8, 16).T.astype(np.int16)      # [16, ntile*8]
    return np.tile(w, (8, 1))                               # [128, ntile*8]


def prep_host(cfg: Cfg, x, src, dst, W1, al1, ar1, b1, W2, al2, ar2, b2):
    """Build per-core input maps."""
    NPC, NCH, T, TA, TB = cfg.NPC, cfg.NCHUNK, cfg.T, cfg.TA, cfg.TB
    SPLIT, GR = cfg.SPLIT, cfg.GR

    # global table row of node v (half-major): half g = loc//GR of core c
    # sits at rows [(g*NCORES + c)*GR, ...) so each half is one contiguous
    # AllGather output and one dma_gather source.
    loc_all = src % NPC
    srcp = ((loc_all // GR) * cfg.NCORES + (src // NPC)) * GR + (loc_all % GR)
    core_of = dst // NPC
    loc = dst % NPC
    chunk_of = loc // P
    dloc = loc % P                                          # dst_local in chunk

    def build_wrhs(W, al, ar):
        blk_l = np.zeros((D, H), np.float32)
        blk_r = np.zeros((D, H), np.float32)
        for h in range(H):
            blk_l[h * DH:(h + 1) * DH, h] = al[h]
            blk_r[h * DH:(h + 1) * DH, h] = ar[h]
        w = np.concatenate([W, W @ blk_l, W @ blk_r], axis=1)  # [256, 264]
        return np.ascontiguousarray(w.reshape(2, P, 264).astype(np.float32))

    w1rhs = build_wrhs(W1, al1, ar1)
    w2rhs = build_wrhs(W2, al2, ar2)
    b1r = np.tile(b1[None, :], (P, 1)).astype(np.float32)
    b2r = np.tile(b2[None, :], (P, 1)).astype(np.float32)
    iota_row = np.tile(np.arange(P, dtype=np.float32)[None, :],
                       (P, 1)).astype(NPBF)

    in_maps = []
    for c in range(cfg.NCORES):
        sel = np.nonzero(core_of == c)[0]
        idxA = np.zeros((P, NCH * TA * 8), np.int16)
        idxB = np.zeros((P, NCH * TB * 8), np.int16)
        mTf = np.zeros((NCH, P, T * P), NPBF)
        mf = np.zeros((NCH, P, T * P), NPBF)
        for k in range(NCH):
            ek = sel[chunk_of[sel] == k]
            sa = srcp[ek]
            dl = dloc[ek]
            a_m = sa < SPLIT
            qa, da = sa[a_m], dl[a_m]
            qb, db = sa[~a_m] - SPLIT, dl[~a_m]
            nA, nB = len(qa), len(qb)
            assert nA <= TA * P, (c, k, nA)
            assert nB <= TB * P, (c, k, nB)
            fa = np.zeros(TA * P, np.int64)      # pads gather row 0
            fa[:nA] = qa
            fb = np.zeros(TB * P, np.int64)
            fb[:nB] = qb
            idxA[:, k * TA * 8:(k + 1) * TA * 8] = _wrap_idx(fa, TA)
            idxB[:, k * TB * 8:(k + 1) * TB * 8] = _wrap_idx(fb, TB)
            # slot (p, t): A edge j=(t*128+p) t<TA ; B edge j=((t-TA)*128+p)
            dcol = np.full((T, P), 128.0, np.float32)
            dcol.reshape(-1)[:nA] = da
            dcol.reshape(-1)[TA * P:TA * P + nB] = db
            # mT[i, e] = (i == dst(e)); m[p, (t, j)] = (dst(t*128+p) == j)
            mTf[k] = (np.arange(P)[:, None] == dcol.reshape(1, -1))
            mf[k] = (dcol.T[:, :, None] ==
                     np.arange(P)[None, None, :]).reshape(P, T * P)
        own = slice(c * NPC, (c + 1) * NPC)
        xTo = np.ascontiguousarray(x[own].T.reshape(2, P, NPC)
                                   .astype(np.float32))
        in_maps.append({
            "xTo": xTo,
            "idxA": idxA, "idxB": idxB,
            "mTf": mTf, "mf": mf,
            "w1rhs": w1rhs, "w2rhs": w2rhs, "b1r": b1r, "b2r": b2r,
            "iota_row": iota_row,
        })
    return in_maps


def assemble_output(cfg: Cfg, x, results):
    h1 = np.concatenate([r["out_h"][:, 0:D] for r in results], axis=0)
    h2 = np.concatenate([r["out_h"][:, D:2 * D] for r in results], axis=0)
    return np.concatenate([x, h1, h2], axis=1)


# ---------------------------------------------------------------- program

def build_program(cfg: Cfg):
    NPC, NCH, T, TA, TB = cfg.NPC, cfg.NCHUNK, cfg.T, cfg.TA, cfg.TB
    NPAD, SPLIT = cfg.NPAD, cfg.SPLIT
    LASTC = cfg.LASTC
    NPCPAD, GR = cfg.NPCPAD, cfg.GR
    CPG = cfg.NCHP // 2                      # chunks per half

    nc = bacc.Bacc("TRN2", target_bir_lowering=False, debug=False,
                   num_devices=cfg.NCORES, num_swdge_queues=2)

    xTo = nc.dram_tensor("xTo", [2, P, NPC], F32, kind="ExternalInput")
    idxA = nc.dram_tensor("idxA", [P, NCH * TA * 8], I16, kind="ExternalInput")
    idxB = nc.dram_tensor("idxB", [P, NCH * TB * 8], I16, kind="ExternalInput")
    mTf = nc.dram_tensor("mTf", [NCH, P, T * P], BF16, kind="ExternalInput")
    mf = nc.dram_tensor("mf", [NCH, P, T * P], BF16, kind="ExternalInput")
    w1rhs = nc.dram_tensor("w1rhs", [2, P, 264], F32, kind="ExternalInput")
    w2rhs = nc.dram_tensor("w2rhs", [2, P, 264], F32, kind="ExternalInput")
    b1r = nc.dram_tensor("b1r", [P, D], F32, kind="ExternalInput")
    b2r = nc.dram_tensor("b2r", [P, D], F32, kind="ExternalInput")
    out_h = nc.dram_tensor("out_h", [NPC, 2 * D], F32, kind="ExternalOutput")

    with tile.TileContext(nc) as tc:
        with tc.tile_pool(name="const", bufs=1) as cp, \
             tc.tile_pool(name="sb", bufs=3) as sb, \
             tc.tile_pool(name="sb2", bufs=2) as sb2, \
             tc.tile_pool(name="sbM", bufs=2) as sbM, \
             tc.tile_pool(name="sbc", bufs=2) as sbc, \
             tc.tile_pool(name="sbT", bufs=2) as sbT, \
             tc.tile_pool(name="ps", bufs=1, space="PSUM") as ps, \
             tc.tile_pool(name="psA", bufs=2, space="PSUM") as psA, \
             tc.tile_pool(name="psR", bufs=2, space="PSUM") as psR, \
             tc.tile_pool(name="psD", bufs=2, space="PSUM") as psD, \
             tc.tile_pool(name="psT", bufs=1, space="PSUM") as psT, \
             tc.tile_pool(name="dram", bufs=1, space="DRAM") as dram:

            tab_own = [dram.tile([NPCPAD, ROWF], F32, tag=f"tab{l}_own",
                                 name=f"tab{l}_own")
                       for l in (1, 2)]
            # one Shared tile per table half (Shared DRAM allows a single
            # writing instruction): [NCORES, GR, ROWF], half-major ids.
            tab_full = [[dram.tile([cfg.NCORES, GR, ROWF], F32,
                                   tag=f"tab{l}{g}_full",
                                   name=f"tab{l}{g}_full",
                                   addr_space="Local" if PROFILE_LOCAL_CC
                                   else "Shared")
                         for g in range(2)]
                        for l in (1, 2)]
            h1T = dram.tile([2, P, NPCPAD], F32, tag="h1T", name="h1T")

            # ---- persistent SBUF ----
            w1_s = cp.tile([P, 2, 264], F32, tag="w1_s")
            w2_s = cp.tile([P, 2, 264], F32, tag="w2_s")
            b1_s = cp.tile([P, D], F32, tag="b1_s")
            b2_s = cp.tile([P, D], F32, tag="b2_s")
            ident_s = cp.tile([P, P], F32, tag="ident_s")
            idxA_s = cp.tile([P, NCH * TA * 8], I16, tag="idxA_s")
            idxB_s = cp.tile([P, NCH * TB * 8], I16, tag="idxB_s")
            er1_s = cp.tile([P, NCH * H], F32, tag="er1_s")
            er2_s = cp.tile([P, NCH * H], F32, tag="er2_s")
            gbuf = [cp.tile([P, T * ROWF], F32, tag=f"G{i}", name=f"G{i}")
                    for i in range(cfg.NG)]

            for d in range(2):
                nc.sync.dma_start(w1_s[:, d, :], w1rhs[d])
                nc.sync.dma_start(w2_s[:, d, :], w2rhs[d])
            nc.sync.dma_start(b1_s[:], b1r[:])
            nc.sync.dma_start(b2_s[:], b2r[:])
            nc.sync.dma_start(idxA_s[:], idxA[:])
            nc.sync.dma_start(idxB_s[:], idxB[:])
            make_identity(nc, ident_s[:])

            def feat_own(w_s, lhsT_dram, l, er_s):
                """Table rows ([feat|el], fp32) + er for OWN nodes; one
                AllGather per CPG-chunk group, pipelined."""
                for k in range(NCH):
                    rows = LASTC if k == NCH - 1 else P
                    xs = sb.tile([P, 2, P], F32, tag="xs", name="xs")
                    for d in range(2):
                        nc.sync.dma_start(
                            xs[:, d, 0:rows],
                            lhsT_dram[d][:, k * P:k * P + rows])
                    f_ps = ps.tile([P, 264], F32, tag="fps")
                    for d in range(2):
                        nc.tensor.matmul(
                            out=f_ps[:rows],
                            lhsT=xs[:, d, 0:rows],
                            rhs=w_s[:, d, :],
                            start=(d == 0), stop=(d == 1))
                    trow = sb.tile([P, RD], F32, tag="trow")
                    nc.scalar.activation(out=trow[:rows],
                                         in_=f_ps[:rows, 0:RD], func=ACT.Copy)
                    nc.scalar.activation(out=er_s[:rows, k * H:(k + 1) * H],
                                         in_=f_ps[:rows, 260:264],
                                         func=ACT.Copy)
                    nc.sync.dma_start(tab_own[l][k * P:k * P + rows, 0:RD],
                                      trow[:rows])
                    if k == CPG - 1 or k == NCH - 1:
                        g = k // CPG
                        if PROFILE_LOCAL_CC:
                            for c in range(cfg.NCORES):
                                nc.sync.dma_start(
                                    tab_full[l][g][c],
                                    tab_own[l][g * GR:(g + 1) * GR, :])
                        else:
                            nc.gpsimd.collective_compute(
                                "AllGather", OP.bypass,
                                replica_groups=[list(range(cfg.NCORES))],
                                ins=[tab_own[l][g * GR:(g + 1) * GR, :]],
                                outs=[tab_full[l][g].opt()])

            def edge_phase(l, er_s, b_s, layer):
                tabA = tab_full[l][0].rearrange("c n f -> (c n) f")
                tabB = tab_full[l][1].rearrange("c n f -> (c n) f")
                for k in range(NCH):
                    rows = LASTC if k == NCH - 1 else P
                    G = gbuf[k % cfg.NG]
                    GA = G[:, 0:TA * ROWF].rearrange("p (t f) -> p t f",
                                                     f=ROWF)
                    GB = G[:, TA * ROWF:T * ROWF].rearrange(
                        "p (t f) -> p t f", f=ROWF)
                    nc.gpsimd.dma_gather(
                        GA, tabA[0:SPLIT, :],
                        idxA_s[:, k * TA * 8:(k + 1) * TA * 8],
                        TA * P, TA * P, ROWF, elem_step=ROWF, queue_num=0,
                        single_packet=False)
                    nc.gpsimd.dma_gather(
                        GB, tabB[0:NPAD - SPLIT, :],
                        idxB_s[:, k * TB * 8:(k + 1) * TB * 8],
                        TB * P, TB * P, ROWF, elem_step=ROWF, queue_num=1,
                        single_packet=False)

                    # match matrices, host-precomputed (bf16):
                    # mT[i, e] = (i == dst(e)); m_all[p, (t, j)]
                    mT = sbT.tile([P, T * P], BF16, tag="mT")
                    nc.sync.dma_start(mT[:], mTf[k])
                    m_all = sbM.tile([P, T * P], BF16, tag="m_all")
                    nc.sync.dma_start(m_all[:], mf[k])

                    # er hi/lo bf16 pair for this chunk's dst nodes
                    erc = er_s[:, k * H:(k + 1) * H]
                    erp = sb.tile([P, 2 * H], BF16, tag="erp")
                    nc.scalar.activation(out=erp[:, 0:H], in_=erc,
                                         func=ACT.Copy)
                    nc.vector.tensor_tensor(out=erp[:, H:2 * H], in0=erc,
                                            in1=erp[:, 0:H], op=OP.subtract)
                    ere_ps = psR.tile([P, T * 2 * H], F32, tag="ere",
                                      name="ere_ps")
                    for t in range(T):
                        nc.tensor.matmul(
                            out=ere_ps[:, t * 2 * H:(t + 1) * 2 * H],
                            lhsT=mT[:, t * P:(t + 1) * P], rhs=erp[:],
                            start=True, stop=True)
                    # s = el + er_hi + er_lo ; exa = exp(leaky_relu(s))
                    ere3 = ere_ps[:].rearrange("p (t f) -> p t f", f=2 * H)
                    sadd = sb2.tile([P, T * H], F32, tag="sadd")
                    sadd3 = sadd[:].rearrange("p (t f) -> p t f", f=H)
                    nc.vector.tensor_tensor(
                        out=sadd3,
                        in0=G[:].rearrange("p (t f) -> p t f",
                                           f=ROWF)[:, :, 256:260],
                        in1=ere3[:, :, 0:H], op=OP.add)
                    nc.vector.tensor_tensor(
                        out=sadd3, in0=sadd3, in1=ere3[:, :, H:2 * H],
                        op=OP.add)
                    # exp(leaky_relu(s)) = max(exp(s), exp(0.2 s)) exactly
                    e1t = sb2.tile([P, T * H], F32, tag="e1t")
                    nc.scalar.activation(out=e1t[:], in_=sadd[:], func=ACT.Exp)
                    e2t = sb2.tile([P, T * H], F32, tag="e2t")
                    nc.scalar.activation(out=e2t[:], in_=sadd[:], func=ACT.Exp,
                                         scale=NEG_SLOPE)
                    exa = sb2.tile([P, T * H], F32, tag="exa")
                    nc.vector.tensor_tensor(out=exa[:], in0=e1t[:],
                                            in1=e2t[:], op=OP.max)
                    # exa hi/lo bf16 pair (denominator needs fp32 grade too:
                    # its error feeds h1 which layer 2 amplifies at the
                    # near-zero tolerance floor)
                    exab = sb2.tile([P, T * 2 * H], BF16, tag="exab")
                    exab3 = exab[:].rearrange("p (t u h) -> p t u h", u=2, h=H)
                    exa3 = exa[:].rearrange("p (t h) -> p t h", h=H)
                    nc.scalar.activation(out=exab3[:, :, 0, :], in_=exa3,
                                         func=ACT.Copy)
                    nc.vector.tensor_tensor(out=exab3[:, :, 1, :], in0=exa3,
                                            in1=exab3[:, :, 0, :],
                                            op=OP.subtract)

                    # C = feat*exa for ALL tiles in one op (f32), then split
                    # into a bf16 hi/lo pair: Cp[:, t] = [Chi_t | Clo_t]
                    TH = T // 2
                    Cp = sbc.tile([P, T * 2 * D], BF16, tag="Cp")
                    Cp3 = Cp[:].rearrange("p (t g) -> p t g", g=2 * D)
                    for u in range(2):
                        ts = slice(u * TH, (u + 1) * TH)
                        Cf = sbc.tile([P, TH * D], F32, tag="Cf")
                        nc.vector.tensor_tensor(
                            out=Cf[:].rearrange("p (t h d) -> p t h d",
                                                h=H, d=DH),
                            in0=G[:].rearrange(
                                "p (t f) -> p t f",
                                f=ROWF)[:, ts, 0:D].rearrange(
                                "p t (h d) -> p t h d", h=H),
                            in1=exa3[:, ts, :, None].to_broadcast(
                                [P, TH, H, DH]),
                            op=OP.mult)
                        nc.scalar.activation(
                            out=Cp3[:, ts, 0:D],
                            in_=Cf[:].rearrange("p (t f) -> p t f", f=D),
                            func=ACT.Copy)
                        nc.vector.tensor_tensor(
                            out=Cp3[:, ts, D:2 * D],
                            in0=Cf[:].rearrange("p (t f) -> p t f", f=D),
                            in1=Cp3[:, ts, 0:D], op=OP.subtract)

                    agg_ps = psA.tile([P, 2 * D], F32, tag="agg")
                    den_ps = psD.tile([P, 2 * H], F32, tag="den",
                                      name="den_ps")
                    for t in range(T):
                        mt = m_all[:, t * P:(t + 1) * P]
                        nc.tensor.matmul(out=agg_ps[:], lhsT=mt,
                                         rhs=Cp[:, t * 2 * D:(t + 1) * 2 * D],
                                         start=(t == 0), stop=(t == T - 1))
                        nc.tensor.matmul(out=den_ps[:], lhsT=mt,
                                         rhs=exab[:, t * 2 * H:(t + 1) * 2 * H],
                                         start=(t == 0), stop=(t == T - 1))
                    # finalize (one PSUM operand per DVE op; hi/lo merges via
                    # scalar-engine staging). +1e-30 guards zero-degree nodes.
                    dsum = sb.tile([P, H], F32, tag="dsum")
                    nc.scalar.activation(out=dsum[:], in_=den_ps[:, 0:H],
                                         func=ACT.Copy, bias=1e-30)
                    den = sb.tile([P, H], F32, tag="den")
                    nc.vector.tensor_tensor(out=den[:], in0=dsum[:],
                                            in1=den_ps[:, H:2 * H], op=OP.add)
                    rden = sb.tile([P, H], F32, tag="rden")
                    nc.vector.reciprocal(rden[:], den[:])
                    hs = sb.tile([P, D], F32, tag="hs")
                    nc.scalar.activation(out=hs[:], in_=agg_ps[:, 0:D],
                                         func=ACT.Copy)
                    nc.vector.tensor_tensor(out=hs[:], in0=hs[:],
                                            in1=agg_ps[:, D:2 * D], op=OP.add)
                    hmat = sb.tile([P, D], F32, tag="hmat")
                    nc.vector.tensor_tensor(
                        out=hmat[:].rearrange("p (h d) -> p h d", h=H),
                        in0=hs[:].rearrange("p (h d) -> p h d", h=H),
                        in1=rden[:, :, None].to_broadcast([P, H, DH]),
                        op=OP.mult)
                    nc.vector.tensor_tensor(out=hmat[:], in0=hmat[:],
                                            in1=b_s[:], op=OP.add)
                    if layer == 1:
                        # elu: e = exp(min(h,0)) via Relu(-h) -> Exp(-r);
                        # h = max(h, e-1)
                        rr = sb.tile([P, D], F32, tag="rr")
                        nc.scalar.activation(out=rr[:], in_=hmat[:],
                                             func=ACT.Relu, scale=-1.0)
                        ee = sb.tile([P, D], F32, tag="ee")
                        nc.scalar.activation(out=ee[:], in_=rr[:],
                                             func=ACT.Exp, scale=-1.0)
                        nc.vector.tensor_scalar_add(ee[:], ee[:], -1.0)
                        nc.vector.tensor_tensor(out=hmat[:], in0=hmat[:],
                                                in1=ee[:], op=OP.max)
                        nc.sync.dma_start(out_h[k * P:k * P + rows, 0:D],
                                          hmat[:rows])
                        for d in range(2):
                            tr_ps = psT.tile([P, P], F32, tag="tr")
                            nc.tensor.transpose(
                                out=tr_ps[:], in_=hmat[:, d * P:(d + 1) * P],
                                identity=ident_s[:])
                            hTt = sb.tile([P, P], F32, tag="hTt")
                            nc.scalar.activation(out=hTt[:], in_=tr_ps[:],
                                                 func=ACT.Copy)
                            nc.sync.dma_start(
                                h1T[d, :, k * P:(k + 1) * P], hTt[:])
                    else:
                        nc.sync.dma_start(out_h[k * P:k * P + rows, D:2 * D],
                                          hmat[:rows])

            # ---------------- layer 1 ----------------
            if _on("feat1"):
                with nc.named_scope("feat1"):
                    feat_own(w1_s, xTo, 0, er1_s)
            if _on("edge1"):
                with nc.named_scope("edge1"):
                    edge_phase(0, er1_s, b1_s, layer=1)

            # ---------------- layer 2 ----------------
            if _on("feat2"):
                with nc.named_scope("feat2"):
                    feat_own(w2_s, h1T, 1, er2_s)
            if _on("edge2"):
                with nc.named_scope("edge2"):
                    edge_phase(1, er2_s, b2_s, layer=2)

    nc.compile()
    return nc


# ------------------------------------------------------------ numpy reference

def ref_numpy(cfg: Cfg, x, src, dst, W1, al1, ar1, b1, W2, al2, ar2, b2):
    def gat(x, W, al, ar, b, elu):
        feat = (x @ W).reshape(cfg.N, H, DH)
        el = np.einsum("nhd,hd->nh", feat, al)
        er = np.einsum("nhd,hd->nh", feat, ar)
        e = el[src] + er[dst]
        e = np.where(e > 0, e, NEG_SLOPE * e)
        ex = np.exp(e)
        denom = np.zeros((cfg.N, H), np.float32)
        np.add.at(denom, dst, ex)
        out = np.zeros((cfg.N, H, DH), np.float32)
        np.add.at(out, dst, feat[src] * (ex / np.maximum(denom[dst], 1e-30))[..., None])
        out = out + b.reshape(1, H, DH)
        if elu:
            out = np.where(out > 0, out, np.exp(np.minimum(out, 0)) - 1)
        return out.reshape(cfg.N, D).astype(np.float32)

    h1 = gat(x, W1, al1, ar1, b1, elu=True)
    h2 = gat(h1, W2, al2, ar2, b2, elu=False)
    return np.concatenate([x, h1, h2], axis=1)


def make_tiny_inputs(cfg: Cfg, seed=0):
    rng = np.random.default_rng(seed)
    x = rng.standard_normal((cfg.N, D), dtype=np.float32)
    src = rng.integers(0, cfg.N, cfg.E).astype(np.int32)
    dst = rng.integers(0, cfg.N, cfg.E).astype(np.int32)
    s1 = 1.0 / np.sqrt(D)
    W1 = (rng.standard_normal((D, D), dtype=np.float32) * s1)
    al1 = (rng.standard_normal((H, DH), dtype=np.float32) * s1)
    ar1 = (rng.standard_normal((H, DH), dtype=np.float32) * s1)
    b1 = np.zeros(D, np.float32)
    W2 = (rng.standard_normal((D, D), dtype=np.float32) * s1)
    al2 = (rng.standard_normal((H, DH), dtype=np.float32) * s1)
    ar2 = (rng.standard_normal((H, DH), dtype=np.float32) * s1)
    b2 = np.zeros(D, np.float32)
    return dict(x=x, src=src, dst=dst, W1=W1, al1=al1, ar1=ar1, b1=b1,
                W2=W2, al2=al2, ar2=ar2, b2=b2)


# ----------------------------- PJRT SPMD runner
import jax
import jax.numpy as jnp
from jax.experimental.shard_map import shard_map
from jax.sharding import Mesh, PartitionSpec

from concourse import bass2jax
from concourse.bass2jax import _bass_exec_p, install_neuronx_cc_hook, partition_id_tensor


class SpmdRunner:
    def __init__(self, nc, n_cores, platform=None):
        install_neuronx_cc_hook()
        self.nc = nc
        self.n_cores = n_cores
        self.platform = platform
        partition_name = (nc.partition_id_tensor.name
                          if nc.partition_id_tensor else None)
        in_names, out_names, out_avals, zero_outs = [], [], [], []
        for alloc in nc.m.functions[0].allocations:
            if not isinstance(alloc, mybir.MemoryLocationSet):
                continue
            name = alloc.memorylocations[0].name
            if alloc.kind == "ExternalInput":
                if name != partition_name:
                    in_names.append(name)
            elif alloc.kind == "ExternalOutput":
                shape = tuple(alloc.tensor_shape)
                dtype = mybir.dt.np(alloc.dtype)
                out_names.append(name)
                out_avals.append(jax.core.ShapedArray(shape, dtype))
                zero_outs.append(np.zeros(shape, dtype))
        self.in_names, self.out_names = in_names, out_names
        self.zero_outs = zero_outs
        n_params = len(in_names)
        n_outs = len(out_avals)
        all_names = list(in_names) + list(out_names)
        if partition_name is not None:
            all_names.append(partition_name)

        def _body(*args):
            operands = list(args)
            if partition_name is not None:
                operands.append(partition_id_tensor())
            outs = _bass_exec_p.bind(
                *operands,
                out_avals=tuple(out_avals),
                in_names=tuple(all_names),
                out_names=tuple(out_names),
                lowering_input_output_aliases=(),
                sim_require_finite=False,
                sim_require_nnan=False,
                nc=nc,
            )
            return tuple(outs)

        devices = jax.devices(platform)[:n_cores]
        self.mesh = Mesh(np.asarray(devices), ("core",))
        in_specs = (PartitionSpec("core"),) * (n_params + n_outs)
        out_specs = (PartitionSpec("core"),) * n_outs
        donate = (() if platform == "cpu" else
                  tuple(range(n_params, n_params + n_outs)))
        self.sharded = jax.jit(
            shard_map(_body, mesh=self.mesh, in_specs=in_specs,
                      out_specs=out_specs, check_rep=False),
            donate_argnums=donate, keep_unused=True)
        self.n_params = n_params
        self.staged = None

    def stage(self, in_maps):
        """Concat per-core inputs and move to devices once."""
        concat = [np.concatenate([np.asarray(in_maps[c][n])
                                  for c in range(self.n_cores)], axis=0)
                  for n in self.in_names]
        sharding = jax.sharding.NamedSharding(self.mesh, PartitionSpec("core"))
        self.staged = [jax.device_put(a, sharding) for a in concat]
        zshapes = [((self.n_cores * z.shape[0],) + z.shape[1:], z.dtype)
                   for z in self.zero_outs]
        self.zero_fn = jax.jit(
            lambda: tuple(jnp.zeros(s, d) for s, d in zshapes),
            out_shardings=tuple(sharding for _ in zshapes))

    def run(self):
        zeros = self.zero_fn()
        jax.block_until_ready(zeros)
        out_arrs = self.sharded(*self.staged, *zeros)
        jax.block_until_ready(out_arrs)
        return out_arrs

    def results(self, out_arrs):
        res = []
        for c in range(self.n_cores):
            d = {}
            for i, name in enumerate(self.out_names):
                full = np.asarray(out_arrs[i])
                per = full.reshape(self.n_cores, -1, *full.shape[1:])[c]
                d[name] = per
            res.append(d)
        return res


# ----------------------------- public entry point

_CACHE = {}


def kernel(x, src, dst, W1, al1, ar1, b1, W2, al2, ar2, b2):
    cfg = FULL
    x = np.asarray(x, np.float32)
    src = np.asarray(src, np.int32)
    dst = np.asarray(dst, np.int32)
    args = [np.asarray(a, np.float32) for a in
            (W1, al1, ar1, b1, W2, al2, ar2, b2)]
    in_maps = prep_host(cfg, x, src, dst, *args)
    if "runner" not in _CACHE:
        nc = build_program(cfg)
        _CACHE["runner"] = SpmdRunner(nc, cfg.NCORES)
    r = _CACHE["runner"]
    r.stage(in_maps)
    out = r.run()
    res = r.results(out)
    return assemble_output(cfg, x, res)
